# revision 2
# baseline (speedup 1.0000x reference)
"""Trainium2 Bass kernel for nn_Agg_57380763075323 (segment_reduce).

Computes, for each (batch, span): [min, max, mean] over the span's tokens of
x[B=16, T=8192, D=256], output [B, S=512, 3*D=768] float32.

Device fast path assumes the uniform span structure produced by
setup_inputs(): span s covers tokens [s*16, (s+1)*16) for all examples.
Anything else falls back to an exact numpy implementation of the reference
semantics (searchsorted-based segment assignment).

Sharding: data-parallel over batch; each of the 8 NeuronCores processes 2
examples. No cross-core communication.

Device algorithm per core (2 examples, each [8192, 256] fp32):
  - View x[b] as [4 tiles, 128 spans, 16 tok * 256 feat]; each tile is one
    contiguous 2MB DMA (16KB per partition row).
  - Per tile, reduce over the 16 tokens with a pairwise log-tree of
    elementwise ops (free-dim slices are token-blocked so level k pairs
    token groups):  max tree + min tree on the Vector engine (DVE),
    sum tree on GPSIMD, mean = sum * (1/16) on the Scalar engine.
  - Results are packed into a [128, 768] tile ([min|max|mean]) and stored
    with one DMA per tile.
"""

import sys

import numpy as np

_TRN_REPO = "/opt/trn_rl_repo"

B, T, D, S = 16, 8192, 256, 512
L = T // S  # 16 tokens per span in the uniform layout
N_CORES = 8
BPC = B // N_CORES  # examples per core
P = 128  # SBUF partitions
TILES = S // P  # span-tiles per example

_PROG_CACHE = {}


def _build_program():
    if _TRN_REPO not in sys.path:
        sys.path.insert(0, _TRN_REPO)
    from concourse import bacc, tile
    import concourse.mybir as mybir

    f32 = mybir.dt.float32
    Alu = mybir.AluOpType

    nc = bacc.Bacc("TRN2", target_bir_lowering=False, debug=False)
    x = nc.dram_tensor("x", [BPC, T, D], f32, kind="ExternalInput").ap()
    out = nc.dram_tensor("out", [BPC, S, 3 * D], f32, kind="ExternalOutput").ap()

    # [BPC, TILES, 128, L*D] — partition rows are whole spans (16KB contiguous)
    xv = x.rearrange("b (i p l) d -> b i p (l d)", i=TILES, p=P, l=L)

    with tile.TileContext(nc) as tc:
        with (
            tc.tile_pool(name="xin", bufs=3) as xin_pool,
            tc.tile_pool(name="scratch", bufs=1) as scratch,
            tc.tile_pool(name="res", bufs=3) as res_pool,
            tc.tile_pool(name="sraw", bufs=2) as sraw_pool,
        ):
            def tree(eng, t, dst, op, tag):
                """Pairwise token-tree reduce of t [128, L*D] into dst [128, D]."""
                cur = t
                w = (L // 2) * D
                while w > D:
                    nxt = scratch.tile([P, w], f32, tag=f"{tag}{w}")
                    eng.tensor_tensor(out=nxt, in0=cur[:, 0:w], in1=cur[:, w : 2 * w], op=op)
                    cur = nxt
                    w //= 2
                eng.tensor_tensor(out=dst, in0=cur[:, 0:D], in1=cur[:, D : 2 * D], op=op)

            for b in range(BPC):
                for i in range(TILES):
                    t = xin_pool.tile([P, L * D], f32, tag="xin")
                    nc.sync.dma_start(out=t, in_=xv[b, i])
                    res = res_pool.tile([P, 3 * D], f32, tag="res")
                    # min | max | mean packed per output row
                    tree(nc.vector, t, res[:, 0:D], Alu.min, "vmin")
                    tree(nc.vector, t, res[:, D : 2 * D], Alu.max, "vmax")
                    sraw = sraw_pool.tile([P, D], f32, tag="sraw")
                    tree(nc.gpsimd, t, sraw, Alu.add, "gsum")
                    nc.scalar.mul(res[:, 2 * D : 3 * D], sraw, 1.0 / L)
                    nc.scalar.dma_start(out=out[b, i * P : (i + 1) * P, :], in_=res)
    nc.compile()
    return nc


def _get_program():
    if "nc" not in _PROG_CACHE:
        _PROG_CACHE["nc"] = _build_program()
    return _PROG_CACHE["nc"]


def _ensure_ntff_hook():
    """Register the axon NTFF profiling hook if the image lacks
    antenv.axon_hooks (replicates trn_boot._ntff_profile_via_ctypes)."""
    try:
        from antenv.axon_hooks import get_axon_ntff_profile_hook  # noqa: F401

        return
    except ImportError:
        pass
    import contextlib
    import ctypes
    import types

    try:
        import antenv
    except ImportError:
        return

    so_path = "/opt/axon/libaxon_pjrt.so"
    mod = types.ModuleType("antenv.axon_hooks")
    holder = {"hook": None}
    mod.set_axon_ntff_profile_hook = lambda h: holder.__setitem__("hook", h)
    mod.get_axon_ntff_profile_hook = lambda: holder["hook"]
    sys.modules["antenv.axon_hooks"] = mod
    antenv.axon_hooks = mod

    try:
        lib = ctypes.CDLL(so_path)
    except OSError:
        return
    if not hasattr(lib, "axon_start_nrt_profile"):
        return
    lib.axon_start_nrt_profile.argtypes = [
        ctypes.POINTER(ctypes.c_int64),
        ctypes.c_size_t,
    ]
    lib.axon_start_nrt_profile.restype = ctypes.c_int64
    lib.axon_stop_nrt_profile.argtypes = [ctypes.c_char_p]
    lib.axon_stop_nrt_profile.restype = ctypes.c_int64

    @contextlib.contextmanager
    def _hook(output_dir, device_ids):
        import jax

        jax.devices()
        if device_ids:
            ids = (ctypes.c_int64 * len(device_ids))(*device_ids)
            rc = lib.axon_start_nrt_profile(ids, len(device_ids))
        else:
            rc = lib.axon_start_nrt_profile(None, 0)
        if rc != 0:
            raise RuntimeError(f"axon_start_nrt_profile rc={rc}")
        try:
            yield
        finally:
            n = lib.axon_stop_nrt_profile(str(output_dir).encode())
            if n < 0:
                raise RuntimeError(f"axon_stop_nrt_profile rc={n}")
            if n == 0:
                print(f"profile: 0 files written to {output_dir}", file=sys.stderr)

    mod.set_axon_ntff_profile_hook(_hook)


def _run_device(x, trace=False):
    """x: [B, T, D] float32 (uniform span layout). Returns ([B, S, 3D], exec_ns)."""
    if _TRN_REPO not in sys.path:
        sys.path.insert(0, _TRN_REPO)
    if trace:
        _ensure_ntff_hook()
    from concourse.bass_utils import run_bass_kernel_spmd

    nc = _get_program()
    in_maps = [
        {"x": np.ascontiguousarray(x[c * BPC : (c + 1) * BPC])} for c in range(N_CORES)
    ]
    res = run_bass_kernel_spmd(
        nc, in_maps, core_ids=list(range(N_CORES)), trace=trace
    )
    out = np.concatenate([res.results[c]["out"] for c in range(N_CORES)], axis=0)
    # Output order per row is [min | max | mean]; reference order is
    # [smin, smax, mean] — identical.
    return out, res.exec_time_ns


def _is_uniform(span_idxs):
    if span_idxs.shape != (B, S, 2):
        return False
    starts = np.arange(S, dtype=np.int64) * L
    return bool(
        np.all(span_idxs[..., 0] == starts[None, :])
        and np.all(span_idxs[..., 1] == starts[None, :] + L)
    )


def _fallback(x, lengths, span_idxs):
    """Exact numpy port of the reference semantics (general spans)."""
    Bn, Tn, Dn = x.shape
    Sn = span_idxs.shape[1]
    starts = span_idxs[..., 0]
    ends = span_idxs[..., 1]
    t = np.arange(Tn)
    out = np.zeros((Bn, Sn, 3 * Dn), np.float32)
    for b in range(Bn):
        seg = np.searchsorted(starts[b], t, side="right") - 1
        seg_c = np.clip(seg, 0, Sn - 1)
        in_span = (seg >= 0) & (t < ends[b][seg_c])
        valid_row = np.arange(Sn) < lengths[b]
        tok_valid = in_span & valid_row[seg_c]
        sid = np.where(tok_valid, seg_c, Sn)
        order = np.argsort(sid, kind="stable")
        ssorted = sid[order]
        xs = x[b][order]
        bounds = np.searchsorted(ssorted, np.arange(Sn + 1))
        for s in range(Sn):
            lo, hi = bounds[s], bounds[s + 1]
            if hi > lo:
                seg_x = xs[lo:hi]
                out[b, s, :Dn] = seg_x.min(axis=0)
                out[b, s, Dn : 2 * Dn] = seg_x.max(axis=0)
                out[b, s, 2 * Dn :] = seg_x.sum(axis=0, dtype=np.float32) / float(
                    hi - lo
                )
    return out


def kernel(x, lengths, span_idxs, _trace=False):
    x = np.asarray(x, dtype=np.float32)
    lengths = np.asarray(lengths, dtype=np.int32)
    span_idxs = np.asarray(span_idxs, dtype=np.int32)

    if x.shape == (B, T, D) and _is_uniform(span_idxs):
        out, exec_ns = _run_device(x, trace=_trace)
        row_ok = np.arange(S)[None, :] < lengths[:, None]
        if not row_ok.all():
            out = np.where(row_ok[..., None], out, np.float32(0.0))
        if _trace:
            return out, exec_ns
        return out

    out = _fallback(x, lengths, span_idxs)
    if _trace:
        return out, None
    return out


if __name__ == "__main__":
    rng = np.random.default_rng(0)
    x = rng.standard_normal((B, T, D), dtype=np.float32)
    starts = (np.arange(S, dtype=np.int32) * L)[None, :].repeat(B, 0)
    span_idxs = np.stack([starts, starts + L], axis=-1).astype(np.int32)
    lengths = np.full((B,), S, dtype=np.int32)
    got = kernel(x, lengths, span_idxs)
    xb = x.reshape(B, S, L, D)
    exp = np.concatenate(
        [xb.min(2), xb.max(2), xb.mean(2, dtype=np.float32)], axis=-1
    )
    err = np.abs(got - exp).max()
    print("self-test max abs err:", err)


# revision 3
# speedup vs baseline: 1.1556x; 1.1556x over previous
"""Trainium2 Bass kernel for nn_Agg_57380763075323 (segment_reduce).

Computes, for each (batch, span): [min, max, mean] over the span's tokens of
x[B=16, T=8192, D=256], output [B, S=512, 3*D=768] float32.

Device fast path assumes the uniform span structure produced by
setup_inputs(): span s covers tokens [s*16, (s+1)*16) for all examples.
Anything else falls back to an exact numpy implementation of the reference
semantics (searchsorted-based segment assignment).

Sharding: data-parallel over batch; each of the 8 NeuronCores processes 2
examples. No cross-core communication.

Device algorithm per core (2 examples, each [8192, 256] fp32):
  - View x[b] as [4 tiles, 128 spans, 16 tok * 256 feat]; each tile is one
    contiguous 2MB DMA (16KB per partition row).
  - Per tile, reduce over the 16 tokens with a pairwise log-tree of
    elementwise ops (free-dim slices are token-blocked so level k pairs
    token groups):  max tree + min tree on the Vector engine (DVE),
    sum tree on GPSIMD, mean = sum * (1/16) on the Scalar engine.
  - Results are packed into a [128, 768] tile ([min|max|mean]) and stored
    with one DMA per tile.
"""

import sys

import numpy as np

_TRN_REPO = "/opt/trn_rl_repo"

B, T, D, S = 16, 8192, 256, 512
L = T // S  # 16 tokens per span in the uniform layout
N_CORES = 8
BPC = B // N_CORES  # examples per core
P = 128  # SBUF partitions
TILES = S // P  # span-tiles per example

_PROG_CACHE = {}


def _build_program():
    if _TRN_REPO not in sys.path:
        sys.path.insert(0, _TRN_REPO)
    from concourse import bacc, tile
    import concourse.mybir as mybir

    f32 = mybir.dt.float32
    Alu = mybir.AluOpType

    nc = bacc.Bacc("TRN2", target_bir_lowering=False, debug=False)
    x = nc.dram_tensor("x", [BPC, T, D], f32, kind="ExternalInput").ap()
    out = nc.dram_tensor("out", [BPC, S, 3 * D], f32, kind="ExternalOutput").ap()

    # [BPC, TILES, 128, L*D] — partition rows are whole spans (16KB contiguous)
    xv = x.rearrange("b (i p l) d -> b i p (l d)", i=TILES, p=P, l=L)

    with tile.TileContext(nc) as tc:
        with (
            tc.tile_pool(name="xin", bufs=3) as xin_pool,
            tc.tile_pool(name="scratch", bufs=1) as scratch,
            tc.tile_pool(name="res", bufs=3) as res_pool,
            tc.tile_pool(name="sraw", bufs=2) as sraw_pool,
        ):
            def tree(eng, t, dst, op, tag):
                """Pairwise token-tree reduce of t [128, L*D] into dst [128, D]."""
                cur = t
                w = (L // 2) * D
                while w > D:
                    nxt = scratch.tile([P, w], f32, tag=f"{tag}{w}")
                    eng.tensor_tensor(out=nxt, in0=cur[:, 0:w], in1=cur[:, w : 2 * w], op=op)
                    cur = nxt
                    w //= 2
                eng.tensor_tensor(out=dst, in0=cur[:, 0:D], in1=cur[:, D : 2 * D], op=op)

            def vreduce(t, dst, op):
                """Single-op DVE reduction over tokens: view tile [P, (t f)]
                as [P, f, t] (inner stride D) and reduce innermost."""
                tv = t.rearrange("p (t f) -> p f t", t=L, f=D)
                nc.vector.tensor_reduce(out=dst, in_=tv, axis=mybir.AxisListType.X, op=op)

            for b in range(BPC):
                for i in range(TILES):
                    t = xin_pool.tile([P, L * D], f32, tag="xin")
                    nc.sync.dma_start(out=t, in_=xv[b, i])
                    res = res_pool.tile([P, 3 * D], f32, tag="res")
                    # min | max | mean packed per output row
                    vreduce(t, res[:, 0:D], Alu.min)
                    vreduce(t, res[:, D : 2 * D], Alu.max)
                    sraw = sraw_pool.tile([P, D], f32, tag="sraw")
                    tree(nc.gpsimd, t, sraw, Alu.add, "gsum")
                    nc.scalar.mul(res[:, 2 * D : 3 * D], sraw, 1.0 / L)
                    nc.scalar.dma_start(out=out[b, i * P : (i + 1) * P, :], in_=res)
    nc.compile()
    return nc


def _get_program():
    if "nc" not in _PROG_CACHE:
        _PROG_CACHE["nc"] = _build_program()
    return _PROG_CACHE["nc"]


def _ensure_ntff_hook():
    """Register the axon NTFF profiling hook if the image lacks
    antenv.axon_hooks (replicates trn_boot._ntff_profile_via_ctypes)."""
    try:
        from antenv.axon_hooks import get_axon_ntff_profile_hook  # noqa: F401

        return
    except ImportError:
        pass
    import contextlib
    import ctypes
    import types

    try:
        import antenv
    except ImportError:
        return

    so_path = "/opt/axon/libaxon_pjrt.so"
    mod = types.ModuleType("antenv.axon_hooks")
    holder = {"hook": None}
    mod.set_axon_ntff_profile_hook = lambda h: holder.__setitem__("hook", h)
    mod.get_axon_ntff_profile_hook = lambda: holder["hook"]
    sys.modules["antenv.axon_hooks"] = mod
    antenv.axon_hooks = mod

    try:
        lib = ctypes.CDLL(so_path)
    except OSError:
        return
    if not hasattr(lib, "axon_start_nrt_profile"):
        return
    lib.axon_start_nrt_profile.argtypes = [
        ctypes.POINTER(ctypes.c_int64),
        ctypes.c_size_t,
    ]
    lib.axon_start_nrt_profile.restype = ctypes.c_int64
    lib.axon_stop_nrt_profile.argtypes = [ctypes.c_char_p]
    lib.axon_stop_nrt_profile.restype = ctypes.c_int64

    @contextlib.contextmanager
    def _hook(output_dir, device_ids):
        import jax

        jax.devices()
        if device_ids:
            ids = (ctypes.c_int64 * len(device_ids))(*device_ids)
            rc = lib.axon_start_nrt_profile(ids, len(device_ids))
        else:
            rc = lib.axon_start_nrt_profile(None, 0)
        if rc != 0:
            raise RuntimeError(f"axon_start_nrt_profile rc={rc}")
        try:
            yield
        finally:
            n = lib.axon_stop_nrt_profile(str(output_dir).encode())
            if n < 0:
                raise RuntimeError(f"axon_stop_nrt_profile rc={n}")
            if n == 0:
                print(f"profile: 0 files written to {output_dir}", file=sys.stderr)

    mod.set_axon_ntff_profile_hook(_hook)


def _run_device(x, trace=False):
    """x: [B, T, D] float32 (uniform span layout). Returns ([B, S, 3D], exec_ns)."""
    if _TRN_REPO not in sys.path:
        sys.path.insert(0, _TRN_REPO)
    if trace:
        _ensure_ntff_hook()
    from concourse.bass_utils import run_bass_kernel_spmd

    nc = _get_program()
    in_maps = [
        {"x": np.ascontiguousarray(x[c * BPC : (c + 1) * BPC])} for c in range(N_CORES)
    ]
    res = run_bass_kernel_spmd(
        nc, in_maps, core_ids=list(range(N_CORES)), trace=trace
    )
    out = np.concatenate([res.results[c]["out"] for c in range(N_CORES)], axis=0)
    # Output order per row is [min | max | mean]; reference order is
    # [smin, smax, mean] — identical.
    return out, res.exec_time_ns


def _is_uniform(span_idxs):
    if span_idxs.shape != (B, S, 2):
        return False
    starts = np.arange(S, dtype=np.int64) * L
    return bool(
        np.all(span_idxs[..., 0] == starts[None, :])
        and np.all(span_idxs[..., 1] == starts[None, :] + L)
    )


def _fallback(x, lengths, span_idxs):
    """Exact numpy port of the reference semantics (general spans)."""
    Bn, Tn, Dn = x.shape
    Sn = span_idxs.shape[1]
    starts = span_idxs[..., 0]
    ends = span_idxs[..., 1]
    t = np.arange(Tn)
    out = np.zeros((Bn, Sn, 3 * Dn), np.float32)
    for b in range(Bn):
        seg = np.searchsorted(starts[b], t, side="right") - 1
        seg_c = np.clip(seg, 0, Sn - 1)
        in_span = (seg >= 0) & (t < ends[b][seg_c])
        valid_row = np.arange(Sn) < lengths[b]
        tok_valid = in_span & valid_row[seg_c]
        sid = np.where(tok_valid, seg_c, Sn)
        order = np.argsort(sid, kind="stable")
        ssorted = sid[order]
        xs = x[b][order]
        bounds = np.searchsorted(ssorted, np.arange(Sn + 1))
        for s in range(Sn):
            lo, hi = bounds[s], bounds[s + 1]
            if hi > lo:
                seg_x = xs[lo:hi]
                out[b, s, :Dn] = seg_x.min(axis=0)
                out[b, s, Dn : 2 * Dn] = seg_x.max(axis=0)
                out[b, s, 2 * Dn :] = seg_x.sum(axis=0, dtype=np.float32) / float(
                    hi - lo
                )
    return out


def kernel(x, lengths, span_idxs, _trace=False):
    x = np.asarray(x, dtype=np.float32)
    lengths = np.asarray(lengths, dtype=np.int32)
    span_idxs = np.asarray(span_idxs, dtype=np.int32)

    if x.shape == (B, T, D) and _is_uniform(span_idxs):
        out, exec_ns = _run_device(x, trace=_trace)
        row_ok = np.arange(S)[None, :] < lengths[:, None]
        if not row_ok.all():
            out = np.where(row_ok[..., None], out, np.float32(0.0))
        if _trace:
            return out, exec_ns
        return out

    out = _fallback(x, lengths, span_idxs)
    if _trace:
        return out, None
    return out


if __name__ == "__main__":
    rng = np.random.default_rng(0)
    x = rng.standard_normal((B, T, D), dtype=np.float32)
    starts = (np.arange(S, dtype=np.int32) * L)[None, :].repeat(B, 0)
    span_idxs = np.stack([starts, starts + L], axis=-1).astype(np.int32)
    lengths = np.full((B,), S, dtype=np.int32)
    got = kernel(x, lengths, span_idxs)
    xb = x.reshape(B, S, L, D)
    exp = np.concatenate(
        [xb.min(2), xb.max(2), xb.mean(2, dtype=np.float32)], axis=-1
    )
    err = np.abs(got - exp).max()
    print("self-test max abs err:", err)


# revision 6
# speedup vs baseline: 1.7063x; 1.4765x over previous
"""Trainium2 Bass kernel for nn_Agg_57380763075323 (segment_reduce).

Computes, for each (batch, span): [min, max, mean] over the span's tokens of
x[B=16, T=8192, D=256], output [B, S=512, 3*D=768] float32.

Device fast path assumes the uniform span structure produced by
setup_inputs(): span s covers tokens [s*16, (s+1)*16) for all examples.
Anything else falls back to an exact numpy implementation of the reference
semantics (searchsorted-based segment assignment).

Sharding: data-parallel over batch; each of the 8 NeuronCores processes 2
examples. No cross-core communication.

Device algorithm per core (2 examples, each [8192, 256] fp32):
  - View x[b] as [4 tiles, 128 spans, 16 tok * 256 feat]; each tile is one
    contiguous 2MB DMA (16KB per partition row).
  - Per tile, reduce over the 16 tokens with a pairwise log-tree of
    elementwise ops (free-dim slices are token-blocked so level k pairs
    token groups):  max tree + min tree on the Vector engine (DVE),
    sum tree on GPSIMD, mean = sum * (1/16) on the Scalar engine.
  - Results are packed into a [128, 768] tile ([min|max|mean]) and stored
    with one DMA per tile.
"""

import sys

import numpy as np

_TRN_REPO = "/opt/trn_rl_repo"

B, T, D, S = 16, 8192, 256, 512
L = T // S  # 16 tokens per span in the uniform layout
N_CORES = 8
BPC = B // N_CORES  # examples per core
P = 128  # SBUF partitions
TILES = S // P  # span-tiles per example

_PROG_CACHE = {}


def _build_program():
    if _TRN_REPO not in sys.path:
        sys.path.insert(0, _TRN_REPO)
    from concourse import bacc, tile
    import concourse.mybir as mybir

    f32 = mybir.dt.float32
    Alu = mybir.AluOpType

    nc = bacc.Bacc("TRN2", target_bir_lowering=False, debug=False)
    x = nc.dram_tensor("x", [BPC, T, D], f32, kind="ExternalInput").ap()
    ident = nc.dram_tensor("ident", [P, P], f32, kind="ExternalInput").ap()
    out = nc.dram_tensor("out", [BPC, S, 3 * D], f32, kind="ExternalOutput").ap()

    # [BPC, TILES, 128, L*D] — partition rows are whole spans (16KB contiguous)
    xv = x.rearrange("b (i p l) d -> b i p (l d)", i=TILES, p=P, l=L)

    with tile.TileContext(nc) as tc:
        with (
            tc.tile_pool(name="xin", bufs=3) as xin_pool,
            tc.tile_pool(name="identp", bufs=1) as ident_pool,
            tc.tile_pool(name="acc", bufs=4, space="PSUM") as acc_pool,
            tc.tile_pool(name="back", bufs=2, space="PSUM") as back_pool,
            tc.tile_pool(name="mid", bufs=3) as mid_pool,
            tc.tile_pool(name="scratch", bufs=1) as scratch,
            tc.tile_pool(name="res", bufs=3) as res_pool,
        ):
            idt = ident_pool.tile([P, P], f32)
            nc.sync.dma_start(out=idt, in_=ident)

            def tree(eng, t, dst, op, tag):
                """Pairwise token-tree reduce of t [128, L*D] into dst [128, D]."""
                cur = t
                w = (L // 2) * D
                while w > D:
                    nxt = scratch.tile([P, w], f32, tag=f"{tag}{w}")
                    eng.tensor_tensor(out=nxt, in0=cur[:, 0:w], in1=cur[:, w : 2 * w], op=op)
                    cur = nxt
                    w //= 2
                eng.tensor_tensor(out=dst, in0=cur[:, 0:D], in1=cur[:, D : 2 * D], op=op)

            def vreduce(t, dst, op):
                """Single-op DVE reduction over tokens: view tile [P, (t f)]
                as [P, f, t] (inner stride D) and reduce innermost."""
                tv = t.rearrange("p (t f) -> p f t", t=L, f=D)
                nc.vector.tensor_reduce(out=dst, in_=tv, axis=mybir.AxisListType.X, op=op)

            for b in range(BPC):
                for i in range(TILES):
                    k = b * TILES + i
                    t = xin_pool.tile([P, L * D], f32, tag="xin")
                    nc.sync.dma_start(out=t, in_=xv[b, i])
                    res = res_pool.tile([P, 3 * D], f32, tag="res")

                    # mean via PE: transpose-accumulate the 16 token chunks
                    # into PSUM ([feat_half, span]), scale on ACT, transpose
                    # back, copy into res.
                    for h in range(2):
                        acc = acc_pool.tile([P, P], f32, tag="acc")
                        for tok in range(L):
                            c = 2 * tok + h
                            nc.tensor.matmul(
                                out=acc,
                                lhsT=t[:, c * P : (c + 1) * P],
                                rhs=idt,
                                is_transpose=True,
                                start=(tok == 0),
                                stop=(tok == L - 1),
                            )
                        mid = mid_pool.tile([P, P], f32, tag="mid")
                        nc.scalar.mul(mid, acc, 1.0 / L)
                        back = back_pool.tile([P, P], f32, tag="back")
                        nc.tensor.matmul(
                            out=back, lhsT=mid, rhs=idt, is_transpose=True,
                            start=True, stop=True,
                        )
                        nc.scalar.copy(
                            out=res[:, 2 * D + h * P : 2 * D + (h + 1) * P], in_=back
                        )

                    # min | max as DVE pairwise trees (GPSIMD TT lacks
                    # min/max opcodes; with GPSIMD idle there's no SBUF-port
                    # contention, so contiguous trees beat strided reduces).
                    tree(nc.vector, t, res[:, 0:D], Alu.min, "vmin")
                    tree(nc.vector, t, res[:, D : 2 * D], Alu.max, "vmax")
                    nc.scalar.dma_start(out=out[b, i * P : (i + 1) * P, :], in_=res)
    nc.compile()
    return nc


def _get_program():
    if "nc" not in _PROG_CACHE:
        _PROG_CACHE["nc"] = _build_program()
    return _PROG_CACHE["nc"]


def _ensure_ntff_hook():
    """Register the axon NTFF profiling hook if the image lacks
    antenv.axon_hooks (replicates trn_boot._ntff_profile_via_ctypes)."""
    try:
        from antenv.axon_hooks import get_axon_ntff_profile_hook  # noqa: F401

        return
    except ImportError:
        pass
    import contextlib
    import ctypes
    import types

    try:
        import antenv
    except ImportError:
        return

    so_path = "/opt/axon/libaxon_pjrt.so"
    mod = types.ModuleType("antenv.axon_hooks")
    holder = {"hook": None}
    mod.set_axon_ntff_profile_hook = lambda h: holder.__setitem__("hook", h)
    mod.get_axon_ntff_profile_hook = lambda: holder["hook"]
    sys.modules["antenv.axon_hooks"] = mod
    antenv.axon_hooks = mod

    try:
        lib = ctypes.CDLL(so_path)
    except OSError:
        return
    if not hasattr(lib, "axon_start_nrt_profile"):
        return
    lib.axon_start_nrt_profile.argtypes = [
        ctypes.POINTER(ctypes.c_int64),
        ctypes.c_size_t,
    ]
    lib.axon_start_nrt_profile.restype = ctypes.c_int64
    lib.axon_stop_nrt_profile.argtypes = [ctypes.c_char_p]
    lib.axon_stop_nrt_profile.restype = ctypes.c_int64

    @contextlib.contextmanager
    def _hook(output_dir, device_ids):
        import jax

        jax.devices()
        if device_ids:
            ids = (ctypes.c_int64 * len(device_ids))(*device_ids)
            rc = lib.axon_start_nrt_profile(ids, len(device_ids))
        else:
            rc = lib.axon_start_nrt_profile(None, 0)
        if rc != 0:
            raise RuntimeError(f"axon_start_nrt_profile rc={rc}")
        try:
            yield
        finally:
            n = lib.axon_stop_nrt_profile(str(output_dir).encode())
            if n < 0:
                raise RuntimeError(f"axon_stop_nrt_profile rc={n}")
            if n == 0:
                print(f"profile: 0 files written to {output_dir}", file=sys.stderr)

    mod.set_axon_ntff_profile_hook(_hook)


def _run_device(x, trace=False):
    """x: [B, T, D] float32 (uniform span layout). Returns ([B, S, 3D], exec_ns)."""
    if _TRN_REPO not in sys.path:
        sys.path.insert(0, _TRN_REPO)
    if trace:
        _ensure_ntff_hook()
    from concourse.bass_utils import run_bass_kernel_spmd

    nc = _get_program()
    ident = np.eye(P, dtype=np.float32)
    in_maps = [
        {"x": np.ascontiguousarray(x[c * BPC : (c + 1) * BPC]), "ident": ident}
        for c in range(N_CORES)
    ]
    res = run_bass_kernel_spmd(
        nc, in_maps, core_ids=list(range(N_CORES)), trace=trace
    )
    out = np.concatenate([res.results[c]["out"] for c in range(N_CORES)], axis=0)
    # Output order per row is [min | max | mean]; reference order is
    # [smin, smax, mean] — identical.
    return out, res.exec_time_ns


def _is_uniform(span_idxs):
    if span_idxs.shape != (B, S, 2):
        return False
    starts = np.arange(S, dtype=np.int64) * L
    return bool(
        np.all(span_idxs[..., 0] == starts[None, :])
        and np.all(span_idxs[..., 1] == starts[None, :] + L)
    )


def _fallback(x, lengths, span_idxs):
    """Exact numpy port of the reference semantics (general spans)."""
    Bn, Tn, Dn = x.shape
    Sn = span_idxs.shape[1]
    starts = span_idxs[..., 0]
    ends = span_idxs[..., 1]
    t = np.arange(Tn)
    out = np.zeros((Bn, Sn, 3 * Dn), np.float32)
    for b in range(Bn):
        seg = np.searchsorted(starts[b], t, side="right") - 1
        seg_c = np.clip(seg, 0, Sn - 1)
        in_span = (seg >= 0) & (t < ends[b][seg_c])
        valid_row = np.arange(Sn) < lengths[b]
        tok_valid = in_span & valid_row[seg_c]
        sid = np.where(tok_valid, seg_c, Sn)
        order = np.argsort(sid, kind="stable")
        ssorted = sid[order]
        xs = x[b][order]
        bounds = np.searchsorted(ssorted, np.arange(Sn + 1))
        for s in range(Sn):
            lo, hi = bounds[s], bounds[s + 1]
            if hi > lo:
                seg_x = xs[lo:hi]
                out[b, s, :Dn] = seg_x.min(axis=0)
                out[b, s, Dn : 2 * Dn] = seg_x.max(axis=0)
                out[b, s, 2 * Dn :] = seg_x.sum(axis=0, dtype=np.float32) / float(
                    hi - lo
                )
    return out


def kernel(x, lengths, span_idxs, _trace=False):
    x = np.asarray(x, dtype=np.float32)
    lengths = np.asarray(lengths, dtype=np.int32)
    span_idxs = np.asarray(span_idxs, dtype=np.int32)

    if x.shape == (B, T, D) and _is_uniform(span_idxs):
        out, exec_ns = _run_device(x, trace=_trace)
        row_ok = np.arange(S)[None, :] < lengths[:, None]
        if not row_ok.all():
            out = np.where(row_ok[..., None], out, np.float32(0.0))
        if _trace:
            return out, exec_ns
        return out

    out = _fallback(x, lengths, span_idxs)
    if _trace:
        return out, None
    return out


if __name__ == "__main__":
    rng = np.random.default_rng(0)
    x = rng.standard_normal((B, T, D), dtype=np.float32)
    starts = (np.arange(S, dtype=np.int32) * L)[None, :].repeat(B, 0)
    span_idxs = np.stack([starts, starts + L], axis=-1).astype(np.int32)
    lengths = np.full((B,), S, dtype=np.int32)
    got = kernel(x, lengths, span_idxs)
    xb = x.reshape(B, S, L, D)
    exp = np.concatenate(
        [xb.min(2), xb.max(2), xb.mean(2, dtype=np.float32)], axis=-1
    )
    err = np.abs(got - exp).max()
    print("self-test max abs err:", err)
